# revision 20
# baseline (speedup 1.0000x reference)
"""Trainium2 Bass kernel for top-2 MoE MLP (SwiGLU experts).

Problem shapes (hardcoded):
  hidden_states [2, 1024, 1024] f32, gate_w [1024, 8] f32,
  w_gate/w_up [8, 1024, 2816] f32, w_down [8, 2816, 1024] f32, top_k = 2.

Strategy: expert-parallel over 8 NeuronCores (1 expert per core).
  - Router (x @ gate_w, softmax, top-2, renorm) computed on host with jax
    on CPU, exactly mirroring the reference implementation.
  - Tokens are dispatched (gathered) to their experts on the host; each
    core computes the SwiGLU MLP for the tokens routed to its expert in
    bf16 (fp32 PSUM accumulation), with all expert weights SBUF-resident.
  - Per-token routing weights are applied on the host during the
    scatter-add combine (fp32).
"""

import numpy as np
import ml_dtypes

import concourse.bacc as bacc
import concourse.tile as tile
import concourse.mybir as mybir
from concourse.bass_utils import run_bass_kernel_spmd

B, S, H = 2, 1024, 1024
E, F, TOPK = 8, 2816, 2
T = B * S
P = 128
F16 = mybir.dt.float16
F32 = mybir.dt.float32

LAST_RESULTS = None  # BassKernelResults of the most recent run (for test harness)

_NC_CACHE = {}


def _to_f16(a: np.ndarray) -> np.ndarray:
    return np.asarray(a, dtype=np.float32).astype(np.float16)


def _routing(x: np.ndarray, gate_w: np.ndarray):
    """Replicates the reference router on CPU jax: softmax fp32, top-2,
    renormalized weights. Returns (sel [T,2] int, top_w [T,2] f32)."""
    import jax
    import jax.numpy as jnp

    cpu = jax.local_devices(backend="cpu")[0]
    with jax.default_device(cpu):
        logits = jnp.asarray(x) @ jnp.asarray(gate_w)
        probs = jax.nn.softmax(logits.astype(jnp.float32), axis=-1)
        top_w, sel = jax.lax.top_k(probs, TOPK)
        top_w = top_w / top_w.sum(axis=-1, keepdims=True)
    return np.asarray(sel), np.asarray(top_w, dtype=np.float32)


def _build_nc(C: int, reps: int = 1):
    """Build the per-core Bass program for capacity-C token batches.

    Inputs (per core): xt [H, C] bf16 (token features, transposed),
      wg/wu [H, F] bf16, wd [F, H] bf16.
    Output: y [C, H] f32 (expert MLP output per gathered token, unscaled).

    reps > 1 repeats the whole computation inside the NEFF (for
    benchmarking by rep-differencing); the result is unchanged.
    """
    nc = bacc.Bacc("TRN2", target_bir_lowering=False, debug=False, num_devices=8)

    xt_d = nc.dram_tensor("xt", [H, C], F16, kind="ExternalInput").ap()
    wg_d = nc.dram_tensor("wg", [H, F], F16, kind="ExternalInput").ap()
    wu_d = nc.dram_tensor("wu", [H, F], F16, kind="ExternalInput").ap()
    wd_d = nc.dram_tensor("wd", [F, H], F16, kind="ExternalInput").ap()
    # output is [H, C] (feature-major); host transposes back
    y_d = nc.dram_tensor("y", [H, C], F32, kind="ExternalOutput").ap()

    NK = H // P   # 8 contraction tiles over H
    NF = F // P   # 22 tiles over F

    with tile.TileContext(nc) as tc:
        with (
            tc.tile_pool(name="weights", bufs=1) as wpool,
            tc.tile_pool(name="acts", bufs=1) as apool,
            tc.tile_pool(name="tmps", bufs=3) as tpool,
            tc.tile_pool(name="outs", bufs=2) as opool,
            tc.tile_pool(name="psum", bufs=2, space="PSUM") as pspool,
        ):
            import contextlib

            def body():
                xt_sb = apool.tile([P, NK, C], F16, tag="xt")
                wg_sb = wpool.tile([P, NK, F], F16, tag="wg")
                wu_sb = wpool.tile([P, NK, F], F16, tag="wu")
                wd_sb = wpool.tile([P, NF, H], F16, tag="wd")

                xt_r = xt_d.rearrange("(ko p) c -> p ko c", p=P)
                wg_r = wg_d.rearrange("(ko p) f -> p ko f", p=P)
                wu_r = wu_d.rearrange("(ko p) f -> p ko f", p=P)
                wd_r = wd_d.rearrange("(fo p) h -> p fo h", p=P)

                # Few large DMAs (per-DMA overhead dominates with many small
                # ones). f-chunked so PE starts early and DMA stays ahead.
                def wchunk(f0, fw):
                    nc.sync.dma_start(wg_sb[:, :, f0:f0 + fw],
                                      wg_r[:, :, f0:f0 + fw])
                    nc.sync.dma_start(wu_sb[:, :, f0:f0 + fw],
                                      wu_r[:, :, f0:f0 + fw])

                nc.sync.dma_start(xt_sb[:, 0:2, :], xt_r[:, 0:2, :])
                wchunk(0, 256)
                nc.sync.dma_start(xt_sb[:, 2:NK, :], xt_r[:, 2:NK, :])
                wchunk(256, 512)
                wchunk(768, 1024)
                wchunk(1792, 1024)
                nc.sync.dma_start(wd_sb[:, 0:NF // 2, :], wd_r[:, 0:NF // 2, :])
                nc.sync.dma_start(wd_sb[:, NF // 2:NF, :], wd_r[:, NF // 2:NF, :])

                ht_sb = apool.tile([P, NF, C], F16, tag="ht")

                # token-chunks along the moving (free) dim, <= 512 each
                chunks = [(c0, min(512, C - c0)) for c0 in range(0, C, 512)]

                # Phase A: g = x @ wg, u = x @ wu, h = silu(g) * u ([F, tok])
                for f in range(NF):
                    pg = pspool.tile([P, C], F32, tag="pg")
                    pu = pspool.tile([P, C], F32, tag="pu")
                    for k in range(NK):
                        lg = wg_sb[:, k, f * P:(f + 1) * P]
                        lu = wu_sb[:, k, f * P:(f + 1) * P]
                        for (c0, cw) in chunks:
                            nc.tensor.matmul(
                                pg[:, c0:c0 + cw], lg, xt_sb[:, k, c0:c0 + cw],
                                start=(k == 0), stop=(k == NK - 1),
                            )
                        for (c0, cw) in chunks:
                            nc.tensor.matmul(
                                pu[:, c0:c0 + cw], lu, xt_sb[:, k, c0:c0 + cw],
                                start=(k == 0), stop=(k == NK - 1),
                            )
                    tmp = tpool.tile([P, C], F32)
                    nc.scalar.activation(
                        tmp[:], pg[:], mybir.ActivationFunctionType.Silu,
                    )
                    nc.vector.tensor_mul(ht_sb[:, f, :], tmp[:], pu[:])

                # Phase B: yT = wd.T @ h  (features on partitions, tokens moving)
                for h0 in range(0, H, P):
                    py = pspool.tile([P, C], F32, tag="pg")
                    for f in range(NF):
                        lw = wd_sb[:, f, h0:h0 + P]
                        for (c0, cw) in chunks:
                            nc.tensor.matmul(
                                py[:, c0:c0 + cw], lw, ht_sb[:, f, c0:c0 + cw],
                                start=(f == 0), stop=(f == NF - 1),
                            )
                    ot = opool.tile([P, C], F32)
                    nc.vector.tensor_copy(ot[:], py[:])
                    nc.sync.dma_start(y_d[h0:h0 + P, :], ot[:])

            if reps == 1:
                body()
            else:
                with tc.For_i(0, reps, 1):
                    body()

    nc.compile()
    return nc


def kernel(hidden_states, gate_w, w_gate, w_up, w_down):
    global LAST_RESULTS

    x = np.ascontiguousarray(np.asarray(hidden_states), dtype=np.float32).reshape(T, H)
    gate_w = np.asarray(gate_w, dtype=np.float32)

    sel, top_w = _routing(x, gate_w)

    # Group (token, slot) pairs by expert.
    flat_sel = sel.ravel()                       # [T*2]
    flat_tok = np.repeat(np.arange(T), TOPK)     # [T*2]
    flat_w = top_w.ravel()                       # [T*2]
    order = np.argsort(flat_sel, kind="stable")
    counts = np.bincount(flat_sel, minlength=E)
    starts = np.concatenate([[0], np.cumsum(counts)])
    toks = [flat_tok[order[starts[e]:starts[e + 1]]] for e in range(E)]
    wts = [flat_w[order[starts[e]:starts[e + 1]]] for e in range(E)]

    C = max(128, int(-(-counts.max() // 64)) * 64)  # capacity, multiple of 64

    # Per-expert transposed, padded, bf16 token batches [H, C].
    xt_all = np.zeros((E, H, C), dtype=np.float16)
    for e in range(E):
        m = counts[e]
        if m:
            xt_all[e, :, :m] = _to_f16(x[toks[e]].T)

    wg_bf = _to_f16(np.asarray(w_gate, dtype=np.float32))
    wu_bf = _to_f16(np.asarray(w_up, dtype=np.float32))
    wd_bf = _to_f16(np.asarray(w_down, dtype=np.float32))

    if C not in _NC_CACHE:
        _NC_CACHE[C] = _build_nc(C, 1)
    nc = _NC_CACHE[C]

    in_maps = [
        {"xt": xt_all[e], "wg": wg_bf[e], "wu": wu_bf[e], "wd": wd_bf[e]}
        for e in range(E)
    ]
    res = run_bass_kernel_spmd(nc, in_maps, core_ids=list(range(E)))
    LAST_RESULTS = res

    out = np.zeros((T, H), dtype=np.float32)
    for e in range(E):
        m = counts[e]
        if m:
            y_e = np.asarray(res.results[e]["y"], dtype=np.float32)[:, :m].T
            out[toks[e]] += wts[e][:, None] * y_e

    return out.reshape(B, S, H)


# revision 23
# speedup vs baseline: 1.0958x; 1.0958x over previous
"""Trainium2 Bass kernel for top-2 MoE MLP (SwiGLU experts).

Problem shapes (hardcoded):
  hidden_states [2, 1024, 1024] f32, gate_w [1024, 8] f32,
  w_gate/w_up [8, 1024, 2816] f32, w_down [8, 2816, 1024] f32, top_k = 2.

Strategy: expert-parallel over 8 NeuronCores (1 expert per core).
  - Router (x @ gate_w, softmax, top-2, renorm) computed on host with jax
    on CPU, exactly mirroring the reference implementation.
  - Tokens are dispatched (gathered) to their experts on the host; each
    core computes the SwiGLU MLP for the tokens routed to its expert in
    fp16 (fp32 PSUM accumulation), with all expert weights SBUF-resident.
  - Per-token routing weights are applied on the host during the
    scatter-add combine (fp32).
"""

import numpy as np

import concourse.bacc as bacc
import concourse.tile as tile
import concourse.mybir as mybir
from concourse.bass_utils import run_bass_kernel_spmd

B, S, H = 2, 1024, 1024
E, F, TOPK = 8, 2816, 2
T = B * S
P = 128
F16 = mybir.dt.float16
F32 = mybir.dt.float32

LAST_RESULTS = None  # BassKernelResults of the most recent run (for test harness)

_NC_CACHE = {}


def _to_f16(a: np.ndarray) -> np.ndarray:
    return np.asarray(a, dtype=np.float32).astype(np.float16)


def _routing(x: np.ndarray, gate_w: np.ndarray):
    """Replicates the reference router on CPU jax: softmax fp32, top-2,
    renormalized weights. Returns (sel [T,2] int, top_w [T,2] f32)."""
    import jax
    import jax.numpy as jnp

    cpu = jax.local_devices(backend="cpu")[0]
    with jax.default_device(cpu):
        logits = jnp.asarray(x) @ jnp.asarray(gate_w)
        probs = jax.nn.softmax(logits.astype(jnp.float32), axis=-1)
        top_w, sel = jax.lax.top_k(probs, TOPK)
        top_w = top_w / top_w.sum(axis=-1, keepdims=True)
    return np.asarray(sel), np.asarray(top_w, dtype=np.float32)


def _build_nc(C: int, reps: int = 1):
    """Build the per-core Bass program for capacity-C token batches.

    Inputs (per core): xt [H, C] fp16 (token features, transposed),
      wg/wu [H, F] fp16, wd [F, H] fp16.
    Output: y [C, H] f32 (expert MLP output per gathered token, unscaled).

    reps > 1 repeats the whole computation inside the NEFF (for
    benchmarking by rep-differencing); the result is unchanged.
    """
    nc = bacc.Bacc("TRN2", target_bir_lowering=False, debug=False, num_devices=8)

    xt_d = nc.dram_tensor("xt", [H, C], F16, kind="ExternalInput").ap()
    wg_d = nc.dram_tensor("wg", [H, F], F16, kind="ExternalInput").ap()
    wu_d = nc.dram_tensor("wu", [H, F], F16, kind="ExternalInput").ap()
    wd_d = nc.dram_tensor("wd", [F, H], F16, kind="ExternalInput").ap()
    # output is [H, C] (feature-major); host transposes back
    y_d = nc.dram_tensor("y", [H, C], F32, kind="ExternalOutput").ap()

    NK = H // P   # 8 contraction tiles over H
    NF = F // P   # 22 tiles over F

    with tile.TileContext(nc) as tc:
        with (
            tc.tile_pool(name="weights", bufs=1) as wpool,
            tc.tile_pool(name="acts", bufs=1) as apool,
            tc.tile_pool(name="tmps", bufs=3) as tpool,
            tc.tile_pool(name="outs", bufs=2) as opool,
            tc.tile_pool(name="psum", bufs=2, space="PSUM") as pspool,
        ):
            def body():
                xt_sb = apool.tile([P, NK, C], F16, tag="xt")
                wg_sb = wpool.tile([P, NK, F], F16, tag="wg")
                wu_sb = wpool.tile([P, NK, F], F16, tag="wu")
                wd_sb = wpool.tile([P, NF, H], F16, tag="wd")

                xt_r = xt_d.rearrange("(ko p) c -> p ko c", p=P)
                wg_r = wg_d.rearrange("(ko p) f -> p ko f", p=P)
                wu_r = wu_d.rearrange("(ko p) f -> p ko f", p=P)
                wd_r = wd_d.rearrange("(fo p) h -> p fo h", p=P)

                # Few large DMAs (per-DMA overhead dominates with many small
                # ones). f-chunked so PE starts early and DMA stays ahead.
                def wchunk(f0, fw):
                    nc.sync.dma_start(wg_sb[:, :, f0:f0 + fw],
                                      wg_r[:, :, f0:f0 + fw])
                    nc.sync.dma_start(wu_sb[:, :, f0:f0 + fw],
                                      wu_r[:, :, f0:f0 + fw])

                nc.sync.dma_start(xt_sb[:, 0:2, :], xt_r[:, 0:2, :])
                wchunk(0, 256)
                nc.sync.dma_start(xt_sb[:, 2:NK, :], xt_r[:, 2:NK, :])
                wchunk(256, 512)
                wchunk(768, 1024)
                wchunk(1792, 1024)
                nc.sync.dma_start(wd_sb[:, 0:NF // 2, :], wd_r[:, 0:NF // 2, :])
                nc.sync.dma_start(wd_sb[:, NF // 2:NF, :], wd_r[:, NF // 2:NF, :])

                ht_sb = apool.tile([P, NF, C], F16, tag="ht")

                # token-chunks along the moving (free) dim, <= 512 each
                chunks = [(c0, min(512, C - c0)) for c0 in range(0, C, 512)]

                # Phase A: g = x @ wg, u = x @ wu, h = silu(g) * u ([F, tok])
                for f in range(NF):
                    pg = pspool.tile([P, C], F32, tag="pg")
                    pu = pspool.tile([P, C], F32, tag="pu")
                    for k in range(NK):
                        lg = wg_sb[:, k, f * P:(f + 1) * P]
                        lu = wu_sb[:, k, f * P:(f + 1) * P]
                        for (c0, cw) in chunks:
                            nc.tensor.matmul(
                                pg[:, c0:c0 + cw], lg, xt_sb[:, k, c0:c0 + cw],
                                start=(k == 0), stop=(k == NK - 1),
                            )
                        for (c0, cw) in chunks:
                            nc.tensor.matmul(
                                pu[:, c0:c0 + cw], lu, xt_sb[:, k, c0:c0 + cw],
                                start=(k == 0), stop=(k == NK - 1),
                            )
                    tmp = tpool.tile([P, C], F32)
                    nc.scalar.activation(
                        tmp[:], pg[:], mybir.ActivationFunctionType.Silu,
                    )
                    nc.vector.tensor_mul(ht_sb[:, f, :], tmp[:], pu[:])

                # Phase B: yT = wd.T @ h  (features on partitions, tokens moving)
                for h0 in range(0, H, P):
                    py = pspool.tile([P, C], F32, tag="pg")
                    for f in range(NF):
                        lw = wd_sb[:, f, h0:h0 + P]
                        for (c0, cw) in chunks:
                            nc.tensor.matmul(
                                py[:, c0:c0 + cw], lw, ht_sb[:, f, c0:c0 + cw],
                                start=(f == 0), stop=(f == NF - 1),
                            )
                    ot = opool.tile([P, C], F32)
                    nc.vector.tensor_copy(ot[:], py[:])
                    nc.sync.dma_start(y_d[h0:h0 + P, :], ot[:])

            if reps == 1:
                body()
            else:
                with tc.For_i(0, reps, 1):
                    body()

    nc.compile()
    return nc


def kernel(hidden_states, gate_w, w_gate, w_up, w_down):
    global LAST_RESULTS

    x = np.ascontiguousarray(np.asarray(hidden_states), dtype=np.float32).reshape(T, H)
    gate_w = np.asarray(gate_w, dtype=np.float32)

    sel, top_w = _routing(x, gate_w)

    # Group (token, slot) pairs by expert.
    flat_sel = sel.ravel()                       # [T*2]
    flat_tok = np.repeat(np.arange(T), TOPK)     # [T*2]
    flat_w = top_w.ravel()                       # [T*2]
    order = np.argsort(flat_sel, kind="stable")
    counts = np.bincount(flat_sel, minlength=E)
    starts = np.concatenate([[0], np.cumsum(counts)])
    toks = [flat_tok[order[starts[e]:starts[e + 1]]] for e in range(E)]
    wts = [flat_w[order[starts[e]:starts[e + 1]]] for e in range(E)]

    C = max(128, int(-(-counts.max() // 64)) * 64)  # capacity, multiple of 64

    # Per-expert transposed, padded, fp16 token batches [H, C].
    xt_all = np.zeros((E, H, C), dtype=np.float16)
    for e in range(E):
        m = counts[e]
        if m:
            xt_all[e, :, :m] = _to_f16(x[toks[e]].T)

    wg_bf = _to_f16(np.asarray(w_gate, dtype=np.float32))
    wu_bf = _to_f16(np.asarray(w_up, dtype=np.float32))
    wd_bf = _to_f16(np.asarray(w_down, dtype=np.float32))

    if C not in _NC_CACHE:
        _NC_CACHE[C] = _build_nc(C, 1)
    nc = _NC_CACHE[C]

    in_maps = [
        {"xt": xt_all[e], "wg": wg_bf[e], "wu": wu_bf[e], "wd": wd_bf[e]}
        for e in range(E)
    ]
    res = run_bass_kernel_spmd(nc, in_maps, core_ids=list(range(E)))
    LAST_RESULTS = res
    globals()["LAST_IN_MAPS"], globals()["LAST_C"] = in_maps, C

    out = np.zeros((T, H), dtype=np.float32)
    for e in range(E):
        m = counts[e]
        if m:
            y_e = np.asarray(res.results[e]["y"], dtype=np.float32)[:, :m].T
            out[toks[e]] += wts[e][:, None] * y_e

    return out.reshape(B, S, H)


# revision 26
# speedup vs baseline: 1.2039x; 1.0987x over previous
"""Trainium2 Bass kernel for top-2 MoE MLP (SwiGLU experts).

Problem shapes (hardcoded):
  hidden_states [2, 1024, 1024] f32, gate_w [1024, 8] f32,
  w_gate/w_up [8, 1024, 2816] f32, w_down [8, 2816, 1024] f32, top_k = 2.

Strategy: expert-parallel over 8 NeuronCores (1 expert per core).
  - Router (x @ gate_w, softmax, top-2, renorm) computed on host with jax
    on CPU, exactly mirroring the reference implementation.
  - Tokens are dispatched (gathered) to their experts on the host; each
    core computes the SwiGLU MLP for the tokens routed to its expert in
    fp16 (fp32 PSUM accumulation), with all expert weights SBUF-resident.
  - Per-token routing weights are applied on the host during the
    scatter-add combine (fp32).
"""

import numpy as np

import concourse.bacc as bacc
import concourse.tile as tile
import concourse.mybir as mybir
from concourse.bass_utils import run_bass_kernel_spmd

B, S, H = 2, 1024, 1024
E, F, TOPK = 8, 2816, 2
T = B * S
P = 128
F16 = mybir.dt.float16
F32 = mybir.dt.float32

LAST_RESULTS = None  # BassKernelResults of the most recent run (for test harness)

_NC_CACHE = {}


def _to_f16(a: np.ndarray) -> np.ndarray:
    return np.asarray(a, dtype=np.float32).astype(np.float16)


def _routing(x: np.ndarray, gate_w: np.ndarray):
    """Replicates the reference router on CPU jax: softmax fp32, top-2,
    renormalized weights. Returns (sel [T,2] int, top_w [T,2] f32)."""
    import jax
    import jax.numpy as jnp

    cpu = jax.local_devices(backend="cpu")[0]
    with jax.default_device(cpu):
        logits = jnp.asarray(x) @ jnp.asarray(gate_w)
        probs = jax.nn.softmax(logits.astype(jnp.float32), axis=-1)
        top_w, sel = jax.lax.top_k(probs, TOPK)
        top_w = top_w / top_w.sum(axis=-1, keepdims=True)
    return np.asarray(sel), np.asarray(top_w, dtype=np.float32)


def _build_nc(C: int, reps: int = 1):
    """Build the per-core Bass program for capacity-C token batches.

    Inputs (per core): xt [H, C] fp16 (token features, transposed),
      wg/wu [H, F] fp16, wd [F, H] fp16.
    Output: y [C, H] f32 (expert MLP output per gathered token, unscaled).

    reps > 1 repeats the whole computation inside the NEFF (for
    benchmarking by rep-differencing); the result is unchanged.
    """
    nc = bacc.Bacc("TRN2", target_bir_lowering=False, debug=False, num_devices=8)

    xt_d = nc.dram_tensor("xt", [H, C], F16, kind="ExternalInput").ap()
    wg_d = nc.dram_tensor("wg", [H, F], F16, kind="ExternalInput").ap()
    wu_d = nc.dram_tensor("wu", [H, F], F16, kind="ExternalInput").ap()
    wd_d = nc.dram_tensor("wd", [F, H], F16, kind="ExternalInput").ap()
    # output is [H, C] (feature-major); host transposes back
    y_d = nc.dram_tensor("y", [H, C], F32, kind="ExternalOutput").ap()

    NK = H // P   # 8 contraction tiles over H
    NF = F // P   # 22 tiles over F

    with tile.TileContext(nc) as tc:
        with (
            tc.tile_pool(name="weights", bufs=1) as wpool,
            tc.tile_pool(name="acts", bufs=1) as apool,
            tc.tile_pool(name="tmps", bufs=3) as tpool,
            tc.tile_pool(name="outs", bufs=2) as opool,
            tc.tile_pool(name="psum", bufs=2, space="PSUM") as pspool,
        ):
            def body():
                xt_sb = apool.tile([P, NK, C], F16, tag="xt")
                wg_sb = wpool.tile([P, NK, F], F16, tag="wg")
                wu_sb = wpool.tile([P, NK, F], F16, tag="wu")
                wd_sb = wpool.tile([P, NF, H], F16, tag="wd")

                xt_r = xt_d.rearrange("(ko p) c -> p ko c", p=P)
                wg_r = wg_d.rearrange("(ko p) f -> p ko f", p=P)
                wu_r = wu_d.rearrange("(ko p) f -> p ko f", p=P)
                wd_r = wd_d.rearrange("(fo p) h -> p fo h", p=P)

                # Few large DMAs (per-DMA overhead dominates with many small
                # ones). f-chunked so PE starts early and DMA stays ahead.
                def wchunk(f0, fw):
                    nc.sync.dma_start(wg_sb[:, :, f0:f0 + fw],
                                      wg_r[:, :, f0:f0 + fw])
                    nc.sync.dma_start(wu_sb[:, :, f0:f0 + fw],
                                      wu_r[:, :, f0:f0 + fw])

                nc.sync.dma_start(xt_sb[:, 0:2, :], xt_r[:, 0:2, :])
                wchunk(0, 256)
                nc.sync.dma_start(xt_sb[:, 2:NK, :], xt_r[:, 2:NK, :])
                wchunk(256, 512)
                wchunk(768, 1024)
                wchunk(1792, 1024)
                nc.sync.dma_start(wd_sb[:, 0:NF // 2, :], wd_r[:, 0:NF // 2, :])
                nc.sync.dma_start(wd_sb[:, NF // 2:NF, :], wd_r[:, NF // 2:NF, :])

                ht_sb = apool.tile([P, NF, C], F16, tag="ht")

                # token-chunks along the moving (free) dim, <= 512 each
                chunks = [(c0, min(512, C - c0)) for c0 in range(0, C, 512)]

                # PE warmup during the initial DMA window: dummy matmuls on
                # an uninitialized scratch tile keep the HAM clock-gate open
                # so the real matmuls start at full rate.
                warm_sb = tpool.tile([P, 512], F16, tag="warm")
                nc.gpsimd.memset(warm_sb[:], 0.0)
                for w in range(16):
                    pw = pspool.tile([P, C], F32, tag="pu")
                    nc.tensor.matmul(pw[:, 0:chunks[0][1]], warm_sb[:, 0:P],
                                     warm_sb[:, 0:chunks[0][1]],
                                     start=True, stop=True)

                # Phase A: g = x @ wg, u = x @ wu, h = silu(g) * u ([F, tok])
                for f in range(NF):
                    pg = pspool.tile([P, C], F32, tag="pg")
                    pu = pspool.tile([P, C], F32, tag="pu")
                    for k in range(NK):
                        lg = wg_sb[:, k, f * P:(f + 1) * P]
                        lu = wu_sb[:, k, f * P:(f + 1) * P]
                        for (c0, cw) in chunks:
                            nc.tensor.matmul(
                                pg[:, c0:c0 + cw], lg, xt_sb[:, k, c0:c0 + cw],
                                start=(k == 0), stop=(k == NK - 1),
                            )
                        for (c0, cw) in chunks:
                            nc.tensor.matmul(
                                pu[:, c0:c0 + cw], lu, xt_sb[:, k, c0:c0 + cw],
                                start=(k == 0), stop=(k == NK - 1),
                            )
                    tmp = tpool.tile([P, C], F32)
                    nc.scalar.activation(
                        tmp[:], pg[:], mybir.ActivationFunctionType.Silu,
                    )
                    nc.vector.tensor_mul(ht_sb[:, f, :], tmp[:], pu[:])

                # Phase B: yT = wd.T @ h  (features on partitions, tokens moving)
                for h0 in range(0, H, P):
                    py = pspool.tile([P, C], F32, tag="pg")
                    for f in range(NF):
                        lw = wd_sb[:, f, h0:h0 + P]
                        for (c0, cw) in chunks:
                            nc.tensor.matmul(
                                py[:, c0:c0 + cw], lw, ht_sb[:, f, c0:c0 + cw],
                                start=(f == 0), stop=(f == NF - 1),
                            )
                    ot = opool.tile([P, C], F32)
                    nc.vector.tensor_copy(ot[:], py[:])
                    nc.sync.dma_start(y_d[h0:h0 + P, :], ot[:])

            if reps == 1:
                body()
            else:
                with tc.For_i(0, reps, 1):
                    body()

    nc.compile()
    return nc


def kernel(hidden_states, gate_w, w_gate, w_up, w_down):
    global LAST_RESULTS

    x = np.ascontiguousarray(np.asarray(hidden_states), dtype=np.float32).reshape(T, H)
    gate_w = np.asarray(gate_w, dtype=np.float32)

    sel, top_w = _routing(x, gate_w)

    # Group (token, slot) pairs by expert.
    flat_sel = sel.ravel()                       # [T*2]
    flat_tok = np.repeat(np.arange(T), TOPK)     # [T*2]
    flat_w = top_w.ravel()                       # [T*2]
    order = np.argsort(flat_sel, kind="stable")
    counts = np.bincount(flat_sel, minlength=E)
    starts = np.concatenate([[0], np.cumsum(counts)])
    toks = [flat_tok[order[starts[e]:starts[e + 1]]] for e in range(E)]
    wts = [flat_w[order[starts[e]:starts[e + 1]]] for e in range(E)]

    C = max(128, int(-(-counts.max() // 8)) * 8)  # capacity, multiple of 8

    # Per-expert transposed, padded, fp16 token batches [H, C].
    xt_all = np.zeros((E, H, C), dtype=np.float16)
    for e in range(E):
        m = counts[e]
        if m:
            xt_all[e, :, :m] = _to_f16(x[toks[e]].T)

    wg_bf = _to_f16(np.asarray(w_gate, dtype=np.float32))
    wu_bf = _to_f16(np.asarray(w_up, dtype=np.float32))
    wd_bf = _to_f16(np.asarray(w_down, dtype=np.float32))

    if C not in _NC_CACHE:
        _NC_CACHE[C] = _build_nc(C, 1)
    nc = _NC_CACHE[C]

    in_maps = [
        {"xt": xt_all[e], "wg": wg_bf[e], "wu": wu_bf[e], "wd": wd_bf[e]}
        for e in range(E)
    ]
    res = run_bass_kernel_spmd(nc, in_maps, core_ids=list(range(E)))
    LAST_RESULTS = res
    globals()["LAST_IN_MAPS"], globals()["LAST_C"] = in_maps, C

    out = np.zeros((T, H), dtype=np.float32)
    for e in range(E):
        m = counts[e]
        if m:
            y_e = np.asarray(res.results[e]["y"], dtype=np.float32)[:, :m].T
            out[toks[e]] += wts[e][:, None] * y_e

    return out.reshape(B, S, H)
